# revision 1
# baseline (speedup 1.0000x reference)
"""Trainium2 Bass kernel for nn_MnistNet (ternary-weight MLP with training-mode
BatchNorm), data-parallel over batch across 8 NeuronCores.

Strategy
--------
- Host side does layout-only prep: transpose x / weights, zero-pad 784->896,
  shard the batch (1024 rows/core) and the weight rows (1/8 per core).
- Device side does all math:
  * ternarize: t = Sign(w/delta - 1) + Sign(w/delta + 1)  in {-2, 0, +2}.
    The 2x scale is exactly washed out by the following BatchNorm (scale
    invariance; eps distortion ~1e-9).  delta = 0.7*mean|W| via per-core
    partial |.| row-sums + one tiny AllReduce + a ones-matmul that both
    finishes the partition reduction and broadcasts the result.
  * biases b_in/b_hid/b_out are dropped entirely: BatchNorm subtracts the
    per-feature mean, so any per-feature constant shift cancels exactly.
  * each core ternarizes 1/8 of each hidden/output weight matrix into a tiled
    bf16 layout and AllGathers the result (weights exactly representable).
  * matmuls in bf16 (fp32 PSUM accumulation), activations kept transposed
    (features on partitions, batch on the free dim) so BN stats are free-dim
    reductions fused into the PSUM-drain ops (accum_out).
  * per-layer BN: 32KB AllReduce of (sum, sumsq), then a fused
    scale/bias ACT pass + min/max DVE clamp (hardtanh).
  * log-softmax: exp on ACT, partition-sum via a ones-matmul, Ln, subtract.
"""

import os

import numpy as np

N_CORES = 8
B = 8192
B_LOC = B // N_CORES          # 1024 rows per core
HID = 4096
N_MID = 4
KIN = 784
KIN_PAD = 896                 # 7 * 128
KT_IN = KIN_PAD // 128        # 7
KT_H = HID // 128             # 32
MT = HID // 128               # 32 output-feature tiles
KL = 4                        # k-tiles per core shard of a hidden layer
SH_H = 512                    # hidden-weight rows per core (4096/8)
EPS = 1e-5
RG = [list(range(N_CORES))]

_cache = {}


def _build():
    n_mid_eff = int(os.environ.get("KERNEL_NMID", str(N_MID)))
    cut = int(os.environ.get("KERNEL_CUT", "0"))
    import concourse.bass as bass
    import concourse.bacc as bacc
    import concourse.mybir as mybir
    import concourse.tile as tile

    f32 = mybir.dt.float32
    bf16 = mybir.dt.bfloat16
    AX = mybir.AxisListType
    OP = mybir.AluOpType
    AF = mybir.ActivationFunctionType

    nc = bacc.Bacc("TRN2", target_bir_lowering=False, debug=False,
                   num_devices=N_CORES)

    # ---- I/O ----------------------------------------------------------------
    xT = nc.dram_tensor("xT", [KIN_PAD, B_LOC], f32, kind="ExternalInput")
    winT = nc.dram_tensor("winT", [KIN_PAD, HID], f32, kind="ExternalInput")
    whT = nc.dram_tensor("whT", [N_MID, SH_H, HID], f32, kind="ExternalInput")
    woT = nc.dram_tensor("woT", [SH_H, 16], f32, kind="ExternalInput")
    gam = nc.dram_tensor("gam", [N_MID + 1, HID], f32, kind="ExternalInput")
    bet = nc.dram_tensor("bet", [N_MID + 1, HID], f32, kind="ExternalInput")
    gob = nc.dram_tensor("gob", [2, 10], f32, kind="ExternalInput")
    # [10, B_LOC] layout: a transposed DRAM store (interleaved partition
    # windows) reliably kills the device, so transpose on host instead.
    out = nc.dram_tensor("out", [10, B_LOC], f32, kind="ExternalOutput")

    with tile.TileContext(nc) as tc:
        with (
            tc.tile_pool(name="ht", bufs=1) as ht,
            tc.tile_pool(name="wmp", bufs=2) as wmp,
            tc.tile_pool(name="slab", bufs=2) as slabp,
            tc.tile_pool(name="tern", bufs=2) as ternp,
            tc.tile_pool(name="sq", bufs=2) as sqp,
            tc.tile_pool(name="small", bufs=1) as small,
            tc.tile_pool(name="stats", bufs=2) as stats,
            tc.tile_pool(name="psum", bufs=2, space="PSUM") as psum,
            tc.tile_pool(name="dram", bufs=1, space="DRAM") as dram,
        ):
            # ---- persistent small tiles ------------------------------------
            ones128 = small.tile([128, 128], f32, tag="ones128")
            nc.vector.memset(ones128, 1.0)
            ones10 = small.tile([10, 1], f32, tag="ones10")
            nc.vector.memset(ones10, 1.0)
            negone = small.tile([128, 1], f32, tag="negone")
            nc.vector.memset(negone, -1.0)
            posone = small.tile([128, 1], f32, tag="posone")
            nc.vector.memset(posone, 1.0)
            epsb = small.tile([128, 1], f32, tag="epsb")
            nc.vector.memset(epsb, EPS)

            gam_sb, bet_sb = [], []
            for l in range(N_MID + 1):
                g = small.tile([128, MT], f32, tag=f"gam{l}", name=f"gam_sb{l}")
                nc.gpsimd.dma_start(out=g, in_=gam[l].rearrange("(m p) -> p m", p=128))
                gam_sb.append(g)
                b = small.tile([128, MT], f32, tag=f"bet{l}", name=f"bet_sb{l}")
                nc.gpsimd.dma_start(out=b, in_=bet[l].rearrange("(m p) -> p m", p=128))
                bet_sb.append(b)
            go_sb = small.tile([10, 1], f32, tag="go")
            nc.gpsimd.dma_start(out=go_sb, in_=gob[0:1, :].rearrange("a f -> f a"))
            bo_sb = small.tile([10, 1], f32, tag="bo")
            nc.gpsimd.dma_start(out=bo_sb, in_=gob[1:2, :].rearrange("a f -> f a"))

            # ---- activation double buffers (transposed: [feat 128, batch]) --
            A = [ht.tile([128, B_LOC], bf16, tag=f"A{k}", name=f"htA{k}")
                 for k in range(KT_H)]
            Bt = [ht.tile([128, B_LOC], bf16, tag=f"B{k}", name=f"htB{k}")
                  for k in range(KT_H)]

            # ---- DRAM scratch ----------------------------------------------
            tw_in = dram.tile([MT, KT_IN, 128, 128], bf16)
            tw_hid_sh = dram.tile([N_MID, MT, KL, 128, 128], bf16)
            tw_hid = [dram.tile([N_CORES, MT, KL, 128, 128], bf16,
                                addr_space="Shared", tag=f"tw_hid{l}",
                                name=f"tw_hid{l}")
                      for l in range(N_MID)]
            tw_out_sh = dram.tile([SH_H, 16], bf16)
            tw_out = dram.tile([HID, 16], bf16, addr_space="Shared")
            dlA_in = dram.tile([128, 1], f32)
            dlA_out = dram.tile([128, 1], f32, addr_space="Shared")
            dlB_in = dram.tile([128, 4], f32)
            dlB_out = dram.tile([128, 4], f32, addr_space="Shared")

            # ---- helpers ----------------------------------------------------
            def bcast_delta(partial_col, n_elems, nm):
                """[128,1] per-partition partial |W| sums -> broadcasted
                1/delta [128,1] (all partitions equal)."""
                ps = psum.tile([128, 1], f32, tag="small", name=f"dps_{nm}",
                               bufs=1)
                nc.tensor.matmul(ps, ones128, partial_col, start=True, stop=True)
                dsc = small.tile([128, 1], f32, tag=f"dsc_{nm}")
                nc.scalar.activation(out=dsc, in_=ps, func=AF.Copy,
                                     scale=0.7 / float(n_elems))
                inv = small.tile([128, 1], f32, tag=f"inv_{nm}")
                nc.vector.reciprocal(out=inv, in_=dsc)
                return inv

            def tern_slab(src_ap, inv_ap, dst_ap, cols):
                """ternarize one [128, cols] f32 slab -> {-2,0,2} bf16 in DRAM.
                dst_ap must be a [128, cols//128, 128] view."""
                sl = slabp.tile([128, cols], f32, tag="slab", name="tslab")
                nc.sync.dma_start(out=sl, in_=src_ap)
                u = ternp.tile([128, cols], bf16, tag="u", name="ternu")
                v = ternp.tile([128, cols], bf16, tag="v", name="ternv")
                nc.scalar.activation(out=u, in_=sl, func=AF.Sign,
                                     bias=negone, scale=inv_ap)
                nc.scalar.activation(out=v, in_=sl, func=AF.Sign,
                                     bias=posone, scale=inv_ap)
                nc.vector.tensor_tensor(out=u, in0=u, in1=v, op=OP.add)
                nc.sync.dma_start(out=dst_ap,
                                  in_=u.rearrange("p (m c) -> p m c", c=128))

            def delta_reduce(src_slabs, n_slabs, nm):
                """abs row-sum partials of a list of slab APs -> [128,1]."""
                part = small.tile([128, 16], f32, tag=f"part_{nm}")
                nc.vector.memset(part, 0.0)
                for s, (ap, cols) in enumerate(src_slabs):
                    sl = slabp.tile([128, cols], f32, tag="slab", name="dslab")
                    nc.sync.dma_start(out=sl, in_=ap)
                    nc.vector.tensor_reduce(out=part[:, s:s + 1], in_=sl,
                                            axis=AX.X, op=OP.add,
                                            apply_absolute_value=True)
                tot = small.tile([128, 1], f32, tag=f"ptot_{nm}")
                nc.vector.tensor_reduce(out=tot, in_=part, axis=AX.X, op=OP.add)
                return tot

            # background work queue: thunks emitted interleaved into m-loops
            bg = []

            def pump(n=1):
                for _ in range(min(n, len(bg))):
                    bg.pop(0)()

            # ---- layer runner ----------------------------------------------
            def mm_layer(lname, ht_in, n_kt, ht_out, w_read, gam_l, bet_l,
                         wm_shape, wm_slice):
                S1 = stats.tile([128, MT], f32, tag="s1", name=f"S1_{lname}")
                S2 = stats.tile([128, MT], f32, tag="s2", name=f"S2_{lname}")
                for m in range(MT):
                    wm = wmp.tile(wm_shape, bf16, tag="wm",
                                  name=f"wm_{lname}_{m}")
                    w_read(m, wm)
                    ps = psum.tile([128, B_LOC], f32, tag="mm",
                                   name=f"ps_{lname}_{m}")
                    for n in range(2):
                        for k in range(n_kt):
                            nc.tensor.matmul(
                                ps[:, n * 512:(n + 1) * 512],
                                wm_slice(wm, k),
                                ht_in[k][:, n * 512:(n + 1) * 512],
                                start=(k == 0), stop=(k == n_kt - 1))
                    nc.vector.tensor_scalar(
                        out=ht_out[m], in0=ps, scalar1=1.0, scalar2=None,
                        op0=OP.mult, op1=OP.add, accum_out=S1[:, m:m + 1])
                    sj = sqp.tile([128, B_LOC], bf16, tag="sq", name="sqj")
                    nc.scalar.activation(out=sj, in_=ps, func=AF.Square,
                                         accum_out=S2[:, m:m + 1])
                    pump(2)
                pump(len(bg))
                # BN stats allreduce
                bin_ = dram.tile([128, 64], f32, tag=f"bns_in_{lname}",
                                 name=f"bns_in_{lname}")
                bout_ = dram.tile([128, 64], f32, addr_space="Shared",
                                  tag=f"bns_out_{lname}", name=f"bns_out_{lname}")
                nc.gpsimd.dma_start(out=bin_[:, 0:32], in_=S1)
                nc.gpsimd.dma_start(out=bin_[:, 32:64], in_=S2)
                nc.gpsimd.collective_compute(
                    "AllReduce", OP.add, replica_groups=RG,
                    ins=[bin_.opt()], outs=[bout_.opt()])
                sg = stats.tile([128, 64], f32, tag="sg", name=f"sg_{lname}")
                nc.gpsimd.dma_start(out=sg, in_=bout_)
                # scale = gamma * rsqrt(var+eps); bias = beta - mean*scale
                mean = stats.tile([128, MT], f32, tag="mean", name=f"mean_{lname}")
                nc.vector.tensor_scalar_mul(mean, sg[:, 0:32], 1.0 / B)
                ex2 = stats.tile([128, MT], f32, tag="ex2", name=f"ex2_{lname}")
                nc.vector.tensor_scalar_mul(ex2, sg[:, 32:64], 1.0 / B)
                msq = stats.tile([128, MT], f32, tag="msq", name=f"msq_{lname}")
                nc.vector.tensor_tensor(out=msq, in0=mean, in1=mean, op=OP.mult)
                var = stats.tile([128, MT], f32, tag="var", name=f"var_{lname}")
                nc.vector.tensor_tensor(out=var, in0=ex2, in1=msq, op=OP.subtract)
                sd = stats.tile([128, MT], f32, tag="sd", name=f"sd_{lname}")
                nc.scalar.activation(out=sd, in_=var, func=AF.Sqrt, bias=epsb)
                rs = stats.tile([128, MT], f32, tag="rs", name=f"rs_{lname}")
                nc.vector.reciprocal(out=rs, in_=sd)
                scl = stats.tile([128, MT], f32, tag="scl", name=f"scl_{lname}")
                nc.vector.tensor_tensor(out=scl, in0=rs, in1=gam_l, op=OP.mult)
                mscl = stats.tile([128, MT], f32, tag="mscl", name=f"mscl_{lname}")
                nc.vector.tensor_tensor(out=mscl, in0=mean, in1=scl, op=OP.mult)
                bia = stats.tile([128, MT], f32, tag="bia", name=f"bia_{lname}")
                nc.vector.tensor_tensor(out=bia, in0=bet_l, in1=mscl,
                                        op=OP.subtract)
                # normalize + hardtanh, in k order for next-layer pipelining
                for k in range(MT):
                    nc.scalar.activation(out=ht_out[k], in_=ht_out[k],
                                         func=AF.Identity,
                                         bias=bia[:, k:k + 1],
                                         scale=scl[:, k:k + 1])
                    nc.vector.tensor_scalar(
                        out=ht_out[k], in0=ht_out[k], scalar1=1.0, scalar2=-1.0,
                        op0=OP.min, op1=OP.max)

            # ================= startup ======================================
            # x load + cast (feeds input-layer matmuls)
            xv = xT.rearrange("(t p) b -> t p b", p=128)
            for k in range(KT_IN):
                xs = slabp.tile([128, B_LOC], f32, tag="slab", name=f"xs{k}")
                nc.sync.dma_start(out=xs, in_=xv[k])
                nc.vector.tensor_copy(out=A[k], in_=xs)

            # delta + ternarize W_in (local, full matrix on every core)
            wv_in = winT.rearrange("(t p) f -> t p f", p=128)
            in_slabs = [(wv_in[t][:, h * 2048:(h + 1) * 2048], 2048)
                        for t in range(KT_IN) for h in range(2)]
            pin = delta_reduce(in_slabs, len(in_slabs), "in")
            inv_in = bcast_delta(pin, KIN * HID, "in")
            for t in range(KT_IN):
                for h in range(2):
                    tern_slab(wv_in[t][:, h * 2048:(h + 1) * 2048], inv_in,
                              tw_in[h * 16:(h + 1) * 16, t].rearrange(
                                  "m p c -> p m c"),
                              2048)

            # delta for hid0 (shard) -> AllReduce #1
            wv_h = [whT[l].rearrange("(kl p) f -> kl p f", p=128)
                    for l in range(N_MID)]
            if n_mid_eff > 0:
                h0_slabs = [(wv_h[0][kl][:, h * 2048:(h + 1) * 2048], 2048)
                            for kl in range(KL) for h in range(2)]
                ph0 = delta_reduce(h0_slabs, len(h0_slabs), "h0")
                nc.gpsimd.dma_start(out=dlA_in, in_=ph0)
                nc.gpsimd.collective_compute(
                    "AllReduce", OP.add, replica_groups=RG,
                    ins=[dlA_in.opt()], outs=[dlA_out.opt()])
                ph0g = small.tile([128, 1], f32, tag="ph0g")
                nc.gpsimd.dma_start(out=ph0g, in_=dlA_out)
                inv_h0 = bcast_delta(ph0g, HID * HID, "h0")

            # ternarize hid0 shard + AllGather (runs during input layer)
            def emit_tern_hid(l, inv):
                for kl in range(KL):
                    for h in range(2):
                        bg.append(lambda l=l, kl=kl, h=h, inv=inv: tern_slab(
                            wv_h[l][kl][:, h * 2048:(h + 1) * 2048], inv,
                            tw_hid_sh[l, h * 16:(h + 1) * 16, kl].rearrange(
                                "m p c -> p m c"),
                            2048))
                bg.append(lambda l=l: nc.gpsimd.collective_compute(
                    "AllGather", OP.bypass, replica_groups=RG,
                    ins=[tw_hid_sh[l].opt()], outs=[tw_hid[l].opt()]))

            if n_mid_eff > 0:
                emit_tern_hid(0, inv_h0)

            # delta partials for hid1..3 + out -> AllReduce #2 (as bg work)
            invs = {}

            def emit_delta_rest():
                pb = small.tile([128, 4], f32, tag="pb")
                nc.vector.memset(pb, 0.0)
                for i, l in enumerate(range(1, n_mid_eff)):
                    slabs = [(wv_h[l][kl][:, h * 2048:(h + 1) * 2048], 2048)
                             for kl in range(KL) for h in range(2)]
                    p = delta_reduce(slabs, len(slabs), f"h{l}")
                    nc.vector.tensor_copy(out=pb[:, i:i + 1], in_=p)
                wv_o = woT.rearrange("(s p) c -> s p c", p=128)
                o_slabs = [(wv_o[s], 16) for s in range(4)]
                po = delta_reduce(o_slabs, len(o_slabs), "out")
                nc.vector.tensor_copy(out=pb[:, 3:4], in_=po)
                nc.gpsimd.dma_start(out=dlB_in, in_=pb)
                nc.gpsimd.collective_compute(
                    "AllReduce", OP.add, replica_groups=RG,
                    ins=[dlB_in.opt()], outs=[dlB_out.opt()])
                pbg = small.tile([128, 4], f32, tag="pbg")
                nc.gpsimd.dma_start(out=pbg, in_=dlB_out)
                for i, l in enumerate(range(1, n_mid_eff)):
                    invs[l] = bcast_delta(pbg[:, i:i + 1], HID * HID, f"h{l}")
                invs["out"] = bcast_delta(pbg[:, 3:4], 10 * HID, "out")

            bg.append(emit_delta_rest)

            # ================= layers =======================================
            def w_read_in(m, wm):
                nc.sync.dma_start(out=wm,
                                  in_=tw_in[m].rearrange("k p c -> p k c"))

            def w_read_hid(l):
                def f(m, wm):
                    # per-rank reads: each is one contiguous 128KB block
                    for r in range(N_CORES):
                        nc.sync.dma_start(
                            out=wm[:, r, :, :],
                            in_=tw_hid[l][r, m].rearrange("kl p c -> p kl c"))
                return f

            bufs = [A, Bt]

            def emit_tern_out():
                wv_o2 = woT.rearrange("(s p) c -> s p c", p=128)
                tv = tw_out_sh.rearrange("(s p) c -> s p c", p=128)
                for s in range(4):
                    sl = slabp.tile([128, 16], f32, tag="slab",
                                    name="oslab")
                    nc.gpsimd.dma_start(out=sl, in_=wv_o2[s])
                    u = ternp.tile([128, 16], bf16, tag="u", name="ou")
                    v = ternp.tile([128, 16], bf16, tag="v", name="ov")
                    nc.scalar.activation(out=u, in_=sl, func=AF.Sign,
                                         bias=negone, scale=invs["out"])
                    nc.scalar.activation(out=v, in_=sl, func=AF.Sign,
                                         bias=posone, scale=invs["out"])
                    nc.vector.tensor_tensor(out=u, in0=u, in1=v, op=OP.add)
                    nc.gpsimd.dma_start(out=tv[s], in_=u)
                nc.gpsimd.collective_compute(
                    "AllGather", OP.bypass, replica_groups=RG,
                    ins=[tw_out_sh.opt()], outs=[tw_out.opt()])

            if n_mid_eff == 0:
                bg.append(emit_tern_out)
            done = False
            if cut == 1:
                pump(len(bg))
                fz = small.tile([10, B_LOC], f32, tag="fz")
                nc.vector.memset(fz, 0.0)
                nc.gpsimd.dma_start(out=out[:], in_=fz)
                done = True
            if not done:
                mm_layer("L0", A, KT_IN, Bt, w_read_in, gam_sb[0],
                         bet_sb[0], [128, KT_IN, 128],
                         lambda wm, k: wm[:, k, :])
            if cut == 2 and not done:
                fz = small.tile([10, B_LOC], f32, tag="fz")
                nc.vector.tensor_copy(out=fz, in_=Bt[0][0:10, :])
                nc.gpsimd.dma_start(out=out[:], in_=fz)
                done = True

            for l in range(n_mid_eff if not done else 0):
                ht_in = bufs[(l + 1) % 2]
                ht_out = bufs[l % 2]
                # queue ternarize of the NEXT hidden layer (or out layer)
                if l + 1 < n_mid_eff:
                    emit_tern_hid(l + 1, invs[l + 1])
                else:
                    bg.append(emit_tern_out)
                mm_layer(f"H{l}", ht_in, KT_H, ht_out, w_read_hid(l),
                         gam_sb[l + 1], bet_sb[l + 1],
                         [128, N_CORES, KL, 128],
                         lambda wm, k: wm[:, k // KL, k % KL, :])

            # ================= output layer + log-softmax ===================
            if not done:
                ht_fin = bufs[(n_mid_eff - 1) % 2]
                wmo = wmp.tile([128, KT_H, 16], bf16, tag="wm", name="wmo")
                nc.sync.dma_start(out=wmo,
                                  in_=tw_out.rearrange("(t p) c -> p t c", p=128))
                pso = psum.tile([10, B_LOC], f32, tag="mm", name="pso")
                for n in range(2):
                    for k in range(KT_H):
                        nc.tensor.matmul(
                            pso[:, n * 512:(n + 1) * 512],
                            wmo[:, k, 0:10],
                            ht_fin[k][:, n * 512:(n + 1) * 512],
                            start=(k == 0), stop=(k == KT_H - 1))
                S1o = stats.tile([10, 1], f32, tag="s1o")
                S2o = stats.tile([10, 1], f32, tag="s2o")
                opre = small.tile([10, B_LOC], f32, tag="opre")
                nc.vector.tensor_scalar(out=opre, in0=pso, scalar1=1.0,
                                        scalar2=None, op0=OP.mult, op1=OP.add,
                                        accum_out=S1o)
                sjo = sqp.tile([10, B_LOC], bf16, tag="sq", name="sqo")
                nc.scalar.activation(out=sjo, in_=pso, func=AF.Square,
                                     accum_out=S2o)
                bno_in = dram.tile([10, 2], f32)
                bno_out = dram.tile([10, 2], f32, addr_space="Shared")
                s12o = stats.tile([10, 2], f32, tag="s12o")
                nc.vector.tensor_copy(out=s12o[:, 0:1], in_=S1o)
                nc.vector.tensor_copy(out=s12o[:, 1:2], in_=S2o)
                nc.gpsimd.dma_start(out=bno_in, in_=s12o)
                nc.gpsimd.collective_compute(
                    "AllReduce", OP.add, replica_groups=RG,
                    ins=[bno_in.opt()], outs=[bno_out.opt()])
                sgo = stats.tile([10, 2], f32, tag="sgo")
                nc.gpsimd.dma_start(out=sgo, in_=bno_out)
                meano = stats.tile([10, 1], f32, tag="meano")
                nc.vector.tensor_scalar_mul(meano, sgo[:, 0:1], 1.0 / B)
                ex2o = stats.tile([10, 1], f32, tag="ex2o")
                nc.vector.tensor_scalar_mul(ex2o, sgo[:, 1:2], 1.0 / B)
                msqo = stats.tile([10, 1], f32, tag="msqo")
                nc.vector.tensor_tensor(out=msqo, in0=meano, in1=meano, op=OP.mult)
                varo = stats.tile([10, 1], f32, tag="varo")
                nc.vector.tensor_tensor(out=varo, in0=ex2o, in1=msqo,
                                        op=OP.subtract)
                sdo = stats.tile([10, 1], f32, tag="sdo")
                nc.scalar.activation(out=sdo, in_=varo, func=AF.Sqrt,
                                     bias=epsb[0:10, :])
                rso = stats.tile([10, 1], f32, tag="rso")
                nc.vector.reciprocal(out=rso, in_=sdo)
                sclo = stats.tile([10, 1], f32, tag="sclo")
                nc.vector.tensor_tensor(out=sclo, in0=rso, in1=go_sb, op=OP.mult)
                mso = stats.tile([10, 1], f32, tag="mso")
                nc.vector.tensor_tensor(out=mso, in0=meano, in1=sclo, op=OP.mult)
                biao = stats.tile([10, 1], f32, tag="biao")
                nc.vector.tensor_tensor(out=biao, in0=bo_sb, in1=mso,
                                        op=OP.subtract)
                onorm = small.tile([10, B_LOC], f32, tag="onorm")
                nc.scalar.activation(out=onorm, in_=opre, func=AF.Identity,
                                     bias=biao, scale=sclo)
                esb = small.tile([10, B_LOC], f32, tag="esb")
                nc.scalar.activation(out=esb, in_=onorm, func=AF.Exp)
                csp = psum.tile([1, B_LOC], f32, tag="cs", bufs=1)
                for n in range(2):
                    nc.tensor.matmul(csp[:, n * 512:(n + 1) * 512], ones10,
                                     esb[:, n * 512:(n + 1) * 512],
                                     start=True, stop=True)
                lsb = small.tile([1, B_LOC], f32, tag="lsb")
                nc.scalar.activation(out=lsb, in_=csp, func=AF.Ln)
                lrow = dram.tile([1, B_LOC], f32)
                nc.gpsimd.dma_start(out=lrow, in_=lsb)
                lr = lrow[0:1, :]
                lb_ap = bass.AP(tensor=lr.tensor, offset=lr.offset,
                                ap=[[0, 10], list(lr.ap[-1])])
                lb = small.tile([10, B_LOC], f32, tag="opre", name="lb")
                nc.gpsimd.dma_start(out=lb, in_=lb_ap)
                fout = small.tile([10, B_LOC], f32, tag="esb", name="fout")
                nc.vector.tensor_tensor(out=fout, in0=onorm, in1=lb,
                                        op=OP.subtract)
                nc.gpsimd.dma_start(out=out[:], in_=fout)

    nc.compile()
    return nc


def _get_program():
    if "nc" not in _cache:
        _cache["nc"] = _build()
    return _cache["nc"]


def kernel(x, W_in, b_in, W_hid, b_hid, W_out, b_out, gamma, beta,
           gamma_out, beta_out):
    from concourse.bass_utils import run_bass_kernel_spmd

    nc = _get_program()

    x = np.asarray(x, dtype=np.float32).reshape(B, KIN)
    # layout-only host prep (transpose + zero-pad + shard)
    xT_full = np.zeros((KIN_PAD, B), dtype=np.float32)
    xT_full[:KIN] = x.T
    winT_full = np.zeros((KIN_PAD, HID), dtype=np.float32)
    winT_full[:KIN] = np.asarray(W_in, dtype=np.float32).T
    whT_full = np.ascontiguousarray(
        np.asarray(W_hid, dtype=np.float32).transpose(0, 2, 1))
    woT_full = np.zeros((HID, 16), dtype=np.float32)
    woT_full[:, :10] = np.asarray(W_out, dtype=np.float32).T
    gam_np = np.ascontiguousarray(np.asarray(gamma, dtype=np.float32))
    bet_np = np.ascontiguousarray(np.asarray(beta, dtype=np.float32))
    gob_np = np.stack([np.asarray(gamma_out, dtype=np.float32),
                       np.asarray(beta_out, dtype=np.float32)])

    in_maps = []
    for c in range(N_CORES):
        in_maps.append({
            "xT": np.ascontiguousarray(
                xT_full[:, c * B_LOC:(c + 1) * B_LOC]),
            "winT": winT_full,
            "whT": np.ascontiguousarray(
                whT_full[:, c * SH_H:(c + 1) * SH_H, :]),
            "woT": np.ascontiguousarray(
                woT_full[c * SH_H:(c + 1) * SH_H, :]),
            "gam": gam_np,
            "bet": bet_np,
            "gob": gob_np,
        })

    res = run_bass_kernel_spmd(nc, in_maps, core_ids=list(range(N_CORES)))
    return np.concatenate(
        [np.ascontiguousarray(res.results[c]["out"].T) for c in range(N_CORES)],
        axis=0)



# revision 11
# speedup vs baseline: 1.4603x; 1.4603x over previous
"""Trainium2 Bass kernel for nn_MnistNet (ternary-weight MLP with training-mode
BatchNorm), data-parallel over batch across 8 NeuronCores.

Strategy (v2, fp8 DoubleRow)
----------------------------
- Host does layout-only prep: transpose x / weights, zero-pad 784->896, shard
  the batch (1024 rows/core) and weight rows (1/8 per core).
- Ternarize on device: t = Sign(w/delta - 1) + Sign(w/delta + 1) in {-2,0,+2}
  (the 2x is washed out exactly by the following BatchNorm). delta needs one
  pass over |W| (per-core partials + tiny AllReduce), ternarize a second pass.
- Hidden-layer matmuls run as fp8e4 DoubleRow (two 128-row contraction slices
  per instruction, measured 2.0x over bf16). Ternary weights are exact in
  fp8; hardtanh activations quantize to e4m3 (rel err ~1.5e-2 through the
  net). Layer 0 (raw x input) and the 10-wide output layer stay bf16 to
  protect accuracy.
- Per-layer weights are ternarized shard-wise, written to DRAM in 4 m-chunks,
  and AllGathered chunk-by-chunk so the next layer can start on early chunks.
- BN stats (sum, sumsq) accumulate in the PSUM-drain ops; the stats AllReduce
  is split in halves (m 0-15 mid-loop, 16-31 at the end) to hide latency.
- Activations live transposed (features on partitions, batch on free dim);
  normalize = ACT affine in-place (bf16) + DVE clamp that also converts to the
  fp8 [128, 2, 1024] pair tiles the next layer's DoubleRow needs.
- log-softmax: Exp on ACT, partition-sum and broadcast via tiny matmuls in
  PSUM, Ln, subtract.
"""

import os

import numpy as np

N_CORES = 8
B = 8192
B_LOC = B // N_CORES          # 1024 rows per core
HID = 4096
N_MID = 4
KIN = 784
KIN_PAD = 896                 # 7 * 128
KT_IN = KIN_PAD // 128        # 7
KT_H = HID // 128             # 32 contraction subtiles
MT = HID // 128               # 32 output-feature tiles
KL = 4                        # k-subtiles per core shard (512/128)
NCH = 4                       # AllGather chunks per hidden layer
MCH = MT // NCH               # 8 m-tiles per chunk
EPS = 1e-5
RG = [list(range(N_CORES))]

_cache = {}


def _build():
    n_mid_eff = int(os.environ.get("KERNEL_NMID", str(N_MID)))
    import concourse.bass as bass
    import concourse.bacc as bacc
    import concourse.mybir as mybir
    import concourse.tile as tile

    f32 = mybir.dt.float32
    bf16 = mybir.dt.bfloat16
    fp8 = mybir.dt.float8e4
    AX = mybir.AxisListType
    OP = mybir.AluOpType
    AF = mybir.ActivationFunctionType
    PM = mybir.MatmulPerfMode

    nc = bacc.Bacc("TRN2", target_bir_lowering=False, debug=False,
                   num_devices=N_CORES)

    # ---- I/O ----------------------------------------------------------------
    xT = nc.dram_tensor("xT", [KIN_PAD, B_LOC], f32, kind="ExternalInput")
    winT = nc.dram_tensor("winT", [KIN_PAD, HID], f32, kind="ExternalInput")
    whT = nc.dram_tensor("whT", [N_MID, 128 * KL, HID], f32,
                         kind="ExternalInput")
    woT = nc.dram_tensor("woT", [128 * KL, 16], f32, kind="ExternalInput")
    gam = nc.dram_tensor("gam", [N_MID + 1, HID], f32, kind="ExternalInput")
    bet = nc.dram_tensor("bet", [N_MID + 1, HID], f32, kind="ExternalInput")
    gob = nc.dram_tensor("gob", [2, 10], f32, kind="ExternalInput")
    out = nc.dram_tensor("out", [10, B_LOC], f32, kind="ExternalOutput")
    dbg_mode = int(os.environ.get("KERNEL_DBG", "0"))
    if dbg_mode:
        dbg = nc.dram_tensor("dbg", [HID, B_LOC], f32, kind="ExternalOutput")

    last_li = n_mid_eff  # layer index (0 = input layer) of the last pre-out layer

    with tile.TileContext(nc) as tc:
        with (
            tc.tile_pool(name="small", bufs=1) as small,
            tc.tile_pool(name="ht", bufs=1) as ht,
            tc.tile_pool(name="prp", bufs=1) as prp,
            tc.tile_pool(name="wmp", bufs=3) as wmp,
            tc.tile_pool(name="slab", bufs=2) as slabp,
            tc.tile_pool(name="tern", bufs=1) as ternp,
            tc.tile_pool(name="sq", bufs=2) as sqp,
            tc.tile_pool(name="stats", bufs=2) as stats,
            tc.tile_pool(name="psum", bufs=2, space="PSUM") as psum,
            tc.tile_pool(name="dram", bufs=1, space="DRAM") as dram,
        ):
            # ---- persistent small tiles ------------------------------------
            ones128 = small.tile([128, 128], f32, tag="ones128")
            nc.vector.memset(ones128, 1.0)
            ones10 = small.tile([10, 1], f32, tag="ones10")
            nc.vector.memset(ones10, 1.0)
            onesr = small.tile([1, 16], f32, tag="onesr")
            nc.vector.memset(onesr, 1.0)
            negone = small.tile([128, 1], f32, tag="negone")
            nc.vector.memset(negone, -1.0)
            posone = small.tile([128, 1], f32, tag="posone")
            nc.vector.memset(posone, 1.0)
            epsb = small.tile([128, 1], f32, tag="epsb")
            nc.vector.memset(epsb, EPS)

            gam_sb, bet_sb = [], []
            for l in range(N_MID + 1):
                g = small.tile([128, MT], f32, tag=f"gam{l}", name=f"gam_sb{l}")
                nc.gpsimd.dma_start(out=g, in_=gam[l].rearrange("(m p) -> p m", p=128))
                gam_sb.append(g)
                b = small.tile([128, MT], f32, tag=f"bet{l}", name=f"bet_sb{l}")
                nc.gpsimd.dma_start(out=b, in_=bet[l].rearrange("(m p) -> p m", p=128))
                bet_sb.append(b)
            go_sb = small.tile([10, 1], f32, tag="go")
            nc.gpsimd.dma_start(out=go_sb, in_=gob[0:1, :].rearrange("a f -> f a"))
            bo_sb = small.tile([10, 1], f32, tag="bo")
            nc.gpsimd.dma_start(out=bo_sb, in_=gob[1:2, :].rearrange("a f -> f a"))

            # ---- activation tiles ------------------------------------------
            # x (L0 moving operand), bf16
            xt = [ht.tile([128, B_LOC], bf16, tag=f"x{t}", name=f"xt{t}")
                  for t in range(KT_IN)]
            # pre-BN activations, bf16, one tile per feature block
            preBN = [ht.tile([128, B_LOC], bf16, tag=f"pb{m}", name=f"pb{m}")
                     for m in range(MT)]
            # fp8 pair tiles for DoubleRow moving operands, two parities
            pairs = [[prp.tile([128, 2, B_LOC], fp8, tag=f"pr{p}_{j}",
                               name=f"pr{p}_{j}") for j in range(MT // 2)]
                     for p in range(2)]

            # ---- DRAM scratch ----------------------------------------------
            tw_in = dram.tile([MT, 128, KT_IN, 128], bf16)
            tw_sh = [[dram.tile([MCH, 128, KL, 128], fp8, tag=f"tsh{l}_{c}",
                                name=f"tw_sh{l}_{c}") for c in range(NCH)]
                     for l in range(N_MID)]
            tw_c = [[dram.tile([N_CORES, MCH, 128, KL, 128], fp8,
                               addr_space="Shared", tag=f"twc{l}_{c}",
                               name=f"tw_c{l}_{c}") for c in range(NCH)]
                    for l in range(N_MID)]
            tw_out_sh = dram.tile([128 * KL, 16], bf16)
            tw_out = dram.tile([HID, 16], bf16, addr_space="Shared")
            dlA_in = dram.tile([128, 1], f32)
            dlA_out = dram.tile([128, 1], f32, addr_space="Shared")
            dlB_in = dram.tile([128, 4], f32)
            dlB_out = dram.tile([128, 4], f32, addr_space="Shared")

            # ---- helpers ----------------------------------------------------
            def bcast_delta(partial_col, n_elems, nm):
                """[128,1] per-partition partial |W| sums -> broadcast 1/delta."""
                ps = psum.tile([128, 1], f32, tag="small", name=f"dps_{nm}",
                               bufs=1)
                nc.tensor.matmul(ps, ones128, partial_col, start=True, stop=True)
                dsc = small.tile([128, 1], f32, tag=f"dsc_{nm}")
                nc.scalar.activation(out=dsc, in_=ps, func=AF.Copy,
                                     scale=0.7 / float(n_elems))
                inv = small.tile([128, 1], f32, tag=f"inv_{nm}")
                nc.vector.reciprocal(out=inv, in_=dsc)
                return inv

            bg = []

            def pump(n=1):
                for _ in range(min(n, len(bg))):
                    bg.pop(0)()

            # ================= startup: x load ===============================
            xv = xT.rearrange("(t p) b -> t p b", p=128)
            for t in range(KT_IN):
                xs = slabp.tile([128, B_LOC], f32, tag="slab", name=f"xs{t}")
                nc.sync.dma_start(out=xs, in_=xv[t])
                nc.vector.tensor_copy(out=xt[t], in_=xs)

            # ================= W_in: delta sweep + ternarize ================
            wv_in = winT.rearrange("(t p) f -> t p f", p=128)
            part_in = small.tile([128, 32], f32, tag="part_in")
            nc.vector.memset(part_in, 0.0)
            for t in range(KT_IN):
                for q in range(4):
                    sl = slabp.tile([128, 1024], f32, tag="slab",
                                    name=f"swin{t}_{q}")
                    nc.sync.dma_start(out=sl,
                                      in_=wv_in[t][:, q * 1024:(q + 1) * 1024])
                    nc.vector.tensor_reduce(out=part_in[:, 4 * t + q:4 * t + q + 1],
                                            in_=sl, axis=AX.X, op=OP.add,
                                            apply_absolute_value=True)
            tot_in = small.tile([128, 1], f32, tag="tot_in")
            nc.vector.tensor_reduce(out=tot_in, in_=part_in, axis=AX.X, op=OP.add)
            inv_in = bcast_delta(tot_in, KIN * HID, "in")

            for q in range(4):
                for t in range(KT_IN):
                    sl = slabp.tile([128, 1024], f32, tag="slab",
                                    name=f"twin{t}_{q}")
                    nc.sync.dma_start(out=sl,
                                      in_=wv_in[t][:, q * 1024:(q + 1) * 1024])
                    u = ternp.tile([128, 1024], bf16, tag="u", name=f"uin{t}{q}")
                    v = ternp.tile([128, 1024], bf16, tag="v", name=f"vin{t}{q}")
                    nc.scalar.activation(out=u, in_=sl, func=AF.Sign,
                                         bias=negone, scale=inv_in)
                    nc.scalar.activation(out=v, in_=sl, func=AF.Sign,
                                         bias=posone, scale=inv_in)
                    nc.vector.tensor_tensor(out=u, in0=u, in1=v, op=OP.add)
                    nc.gpsimd.dma_start(
                        out=tw_in[q * 8:(q + 1) * 8, :, t, :].rearrange(
                            "m p c -> p m c"),
                        in_=u.rearrange("p (m c) -> p m c", c=128))

            # ================= hidden deltas ================================
            wv_h = [whT[l].rearrange("(kl p) f -> kl p f", p=128)
                    for l in range(N_MID)]

            def sweep_hid(l, nm):
                part = small.tile([128, 16], f32, tag=f"part_{nm}")
                nc.vector.memset(part, 0.0)
                for kl in range(KL):
                    for c in range(NCH):
                        sl = slabp.tile([128, 1024], f32, tag="slab",
                                        name=f"sw{nm}_{kl}_{c}")
                        nc.gpsimd.dma_start(
                            out=sl, in_=wv_h[l][kl][:, c * 1024:(c + 1) * 1024])
                        nc.vector.tensor_reduce(
                            out=part[:, kl * NCH + c:kl * NCH + c + 1], in_=sl,
                            axis=AX.X, op=OP.add, apply_absolute_value=True)
                tot = small.tile([128, 1], f32, tag=f"tot_{nm}")
                nc.vector.tensor_reduce(out=tot, in_=part, axis=AX.X, op=OP.add)
                return tot

            invs = {}
            if n_mid_eff > 0:
                ph0 = sweep_hid(0, "h0")
                nc.gpsimd.dma_start(out=dlA_in, in_=ph0)
                nc.gpsimd.collective_compute(
                    "AllReduce", OP.add, replica_groups=RG,
                    ins=[dlA_in.opt()], outs=[dlA_out.opt()])
                ph0g = small.tile([128, 1], f32, tag="ph0g")
                nc.gpsimd.dma_start(out=ph0g, in_=dlA_out)
                invs[0] = bcast_delta(ph0g, HID * HID, "h0")

            # ---- ternarize one hidden layer (chunked, queued as bg work) ----
            def emit_tern_hid(l, inv):
                for c in range(NCH):
                    for kl in range(KL):
                        def tern1(l=l, c=c, kl=kl, inv=inv):
                            sl = slabp.tile([128, 1024], f32, tag="slab",
                                            name=f"tsl{l}_{c}_{kl}")
                            nc.gpsimd.dma_start(
                                out=sl,
                                in_=wv_h[l][kl][:, c * 1024:(c + 1) * 1024])
                            u = ternp.tile([128, 1024], fp8, tag="u",
                                           name=f"u{l}_{c}_{kl}")
                            v = ternp.tile([128, 1024], fp8, tag="v",
                                           name=f"v{l}_{c}_{kl}")
                            nc.scalar.activation(out=u, in_=sl, func=AF.Sign,
                                                 bias=negone, scale=inv)
                            nc.scalar.activation(out=v, in_=sl, func=AF.Sign,
                                                 bias=posone, scale=inv)
                            nc.vector.tensor_tensor(out=u, in0=u, in1=v,
                                                    op=OP.add)
                            nc.gpsimd.dma_start(
                                out=tw_sh[l][c][:, :, kl, :].rearrange(
                                    "m p c -> p m c"),
                                in_=u.rearrange("p (m c) -> p m c", c=128))
                        bg.append(tern1)

                    def ag1(l=l, c=c):
                        nc.gpsimd.collective_compute(
                            "AllGather", OP.bypass, replica_groups=RG,
                            ins=[tw_sh[l][c].opt()], outs=[tw_c[l][c].opt()])
                    bg.append(ag1)

            if n_mid_eff > 0:
                emit_tern_hid(0, invs[0])

            # ---- deltas for hid1..3 + out, one AllReduce (bg work) ---------
            def emit_delta_rest():
                tots = {}
                for l in range(1, n_mid_eff):
                    def sw(l=l):
                        tots[l] = sweep_hid(l, f"h{l}")
                    bg.append(sw)

                def sw_out():
                    wv_o = woT.rearrange("(s p) c -> s p c", p=128)
                    part = small.tile([128, 4], f32, tag="part_out")
                    nc.vector.memset(part, 0.0)
                    for s in range(4):
                        sl = slabp.tile([128, 16], f32, tag="oslab",
                                        name=f"swo{s}")
                        nc.gpsimd.dma_start(out=sl, in_=wv_o[s])
                        nc.vector.tensor_reduce(out=part[:, s:s + 1], in_=sl,
                                                axis=AX.X, op=OP.add,
                                                apply_absolute_value=True)
                    tot = small.tile([128, 1], f32, tag="tot_out")
                    nc.vector.tensor_reduce(out=tot, in_=part, axis=AX.X,
                                            op=OP.add)
                    tots["out"] = tot
                bg.append(sw_out)

                def ar2():
                    pb = small.tile([128, 4], f32, tag="pb")
                    nc.vector.memset(pb, 0.0)
                    for i, l in enumerate(range(1, n_mid_eff)):
                        nc.vector.tensor_copy(out=pb[:, i:i + 1], in_=tots[l])
                    nc.vector.tensor_copy(out=pb[:, 3:4], in_=tots["out"])
                    nc.gpsimd.dma_start(out=dlB_in, in_=pb)
                    nc.gpsimd.collective_compute(
                        "AllReduce", OP.add, replica_groups=RG,
                        ins=[dlB_in.opt()], outs=[dlB_out.opt()])
                    pbg = small.tile([128, 4], f32, tag="pbg")
                    nc.gpsimd.dma_start(out=pbg, in_=dlB_out)
                    for i, l in enumerate(range(1, n_mid_eff)):
                        invs[l] = bcast_delta(pbg[:, i:i + 1], HID * HID,
                                              f"h{l}")
                    invs["out"] = bcast_delta(pbg[:, 3:4], 10 * HID, "out")
                return ar2

            ar2 = emit_delta_rest()

            # ---- out-layer ternarize (bf16) --------------------------------
            def emit_tern_out():
                wv_o = woT.rearrange("(s p) c -> s p c", p=128)
                tv = tw_out_sh.rearrange("(s p) c -> s p c", p=128)
                for s in range(4):
                    sl = slabp.tile([128, 16], f32, tag="oslab", name=f"osl{s}")
                    nc.gpsimd.dma_start(out=sl, in_=wv_o[s])
                    u = ternp.tile([128, 16], bf16, tag="u", name=f"ou{s}")
                    v = ternp.tile([128, 16], bf16, tag="v", name=f"ov{s}")
                    nc.scalar.activation(out=u, in_=sl, func=AF.Sign,
                                         bias=negone, scale=invs["out"])
                    nc.scalar.activation(out=v, in_=sl, func=AF.Sign,
                                         bias=posone, scale=invs["out"])
                    nc.vector.tensor_tensor(out=u, in0=u, in1=v, op=OP.add)
                    nc.gpsimd.dma_start(out=tv[s], in_=u)
                nc.gpsimd.collective_compute(
                    "AllGather", OP.bypass, replica_groups=RG,
                    ins=[tw_out_sh.opt()], outs=[tw_out.opt()])

            # ---- BN stats + normalize --------------------------------------
            def stage_stats_half(lname, S1h, S2h, half):
                bin_ = dram.tile([128, 32], f32, tag=f"bsi_{lname}{half}",
                                 name=f"bsi_{lname}{half}")
                bout = dram.tile([128, 32], f32, addr_space="Shared",
                                 tag=f"bso_{lname}{half}",
                                 name=f"bso_{lname}{half}")
                nc.gpsimd.dma_start(out=bin_[:, 0:16], in_=S1h)
                nc.gpsimd.dma_start(out=bin_[:, 16:32], in_=S2h)
                nc.gpsimd.collective_compute(
                    "AllReduce", OP.add, replica_groups=RG,
                    ins=[bin_.opt()], outs=[bout.opt()])
                return bout

            def bn_normalize(lname, bouts, gam_l, bet_l, out_parity):
                sclh, biah = [], []
                for half in range(2):
                    sg = stats.tile([128, 32], f32, tag="sg",
                                    name=f"sg_{lname}{half}")
                    nc.gpsimd.dma_start(out=sg, in_=bouts[half])
                    mean = stats.tile([128, 16], f32, tag="mean",
                                      name=f"mean_{lname}{half}")
                    nc.vector.tensor_scalar_mul(mean, sg[:, 0:16], 1.0 / B)
                    ex2 = stats.tile([128, 16], f32, tag="ex2",
                                     name=f"ex2_{lname}{half}")
                    nc.vector.tensor_scalar_mul(ex2, sg[:, 16:32], 1.0 / B)
                    msq = stats.tile([128, 16], f32, tag="msq",
                                     name=f"msq_{lname}{half}")
                    nc.vector.tensor_tensor(out=msq, in0=mean, in1=mean,
                                            op=OP.mult)
                    var = stats.tile([128, 16], f32, tag="var",
                                     name=f"var_{lname}{half}")
                    nc.vector.tensor_tensor(out=var, in0=ex2, in1=msq,
                                            op=OP.subtract)
                    sd = stats.tile([128, 16], f32, tag="sd",
                                    name=f"sd_{lname}{half}")
                    nc.scalar.activation(out=sd, in_=var, func=AF.Sqrt,
                                         bias=epsb)
                    rs = stats.tile([128, 16], f32, tag="rs",
                                    name=f"rs_{lname}{half}")
                    nc.vector.reciprocal(out=rs, in_=sd)
                    scl = stats.tile([128, 16], f32, tag="scl",
                                     name=f"scl_{lname}{half}")
                    sl = slice(16 * half, 16 * half + 16)
                    nc.vector.tensor_tensor(out=scl, in0=rs, in1=gam_l[:, sl],
                                            op=OP.mult)
                    mscl = stats.tile([128, 16], f32, tag="mscl",
                                      name=f"mscl_{lname}{half}")
                    nc.vector.tensor_tensor(out=mscl, in0=mean, in1=scl,
                                            op=OP.mult)
                    bia = stats.tile([128, 16], f32, tag="bia",
                                     name=f"bia_{lname}{half}")
                    nc.vector.tensor_tensor(out=bia, in0=bet_l[:, sl], in1=mscl,
                                            op=OP.subtract)
                    sclh.append(scl)
                    biah.append(bia)
                for m in range(MT):
                    h, i = divmod(m, 16)
                    nc.scalar.activation(out=preBN[m], in_=preBN[m],
                                         func=AF.Identity,
                                         bias=biah[h][:, i:i + 1],
                                         scale=sclh[h][:, i:i + 1])
                    if out_parity is None:
                        nc.vector.tensor_scalar(
                            out=preBN[m], in0=preBN[m], scalar1=1.0,
                            scalar2=-1.0, op0=OP.min, op1=OP.max)
                    else:
                        nc.vector.tensor_scalar(
                            out=pairs[out_parity][m // 2][:, m % 2, :],
                            in0=preBN[m], scalar1=1.0, scalar2=-1.0,
                            op0=OP.min, op1=OP.max)

            # ---- generic layer runner --------------------------------------
            def run_layer(li, lname, gam_l, bet_l):
                """li: layer index, 0 = input layer, 1..N_MID = hidden."""
                S1h = [stats.tile([128, 16], f32, tag=f"s1h{h}",
                                  name=f"S1{h}_{lname}") for h in range(2)]
                S2h = [stats.tile([128, 16], f32, tag=f"s2h{h}",
                                  name=f"S2{h}_{lname}") for h in range(2)]
                bouts = {}
                pairs_in = pairs[(li - 1) % 2] if li > 0 else None
                for m in range(MT):
                    ps = psum.tile([128, B_LOC], f32, tag="mm",
                                   name=f"ps_{lname}_{m}")
                    if li == 0:
                        wmi = wmp.tile([128, KT_IN, 128], bf16, tag="wmi",
                                       name=f"wmi_{m}", bufs=2)
                        nc.sync.dma_start(out=wmi, in_=tw_in[m])
                        for t in range(KT_IN):
                            for n in range(2):
                                nc.tensor.matmul(
                                    ps[:, n * 512:(n + 1) * 512],
                                    wmi[:, t, :], xt[t][:, n * 512:(n + 1) * 512],
                                    start=(t == 0), stop=(t == KT_IN - 1))
                    else:
                        c, mr = divmod(m, MCH)
                        wm = wmp.tile([128, KT_H, 128], fp8, tag="wm",
                                      name=f"wm_{lname}_{m}")
                        for r in range(N_CORES):
                            nc.sync.dma_start(out=wm[:, KL * r:KL * r + KL, :],
                                              in_=tw_c[li - 1][c][r, mr])
                        for j in range(KT_H // 2):
                            for n in range(2):
                                nc.tensor.matmul(
                                    ps[:, n * 512:(n + 1) * 512],
                                    wm[:, 2 * j:2 * j + 2, :],
                                    pairs_in[j][:, :, n * 512:(n + 1) * 512],
                                    start=(j == 0), stop=(j == KT_H // 2 - 1),
                                    perf_mode=PM.DoubleRow)
                    h, i = divmod(m, 16)
                    nc.vector.tensor_scalar(
                        out=preBN[m], in0=ps, scalar1=1.0, scalar2=None,
                        op0=OP.mult, op1=OP.add, accum_out=S1h[h][:, i:i + 1])
                    sj = sqp.tile([128, B_LOC], bf16, tag="sq",
                                  name=f"sq_{lname}_{m}")
                    nc.scalar.activation(out=sj, in_=ps, func=AF.Square,
                                         accum_out=S2h[h][:, i:i + 1])
                    if m == 15:
                        bouts[0] = stage_stats_half(lname, S1h[0], S2h[0], 0)
                    pump(2)
                bouts[1] = stage_stats_half(lname, S1h[1], S2h[1], 1)
                if dbg_mode == 2 and li == 0:
                    dv = dbg.rearrange("(m p) b -> m p b", p=128)
                    for m in range(MT):
                        dsl = slabp.tile([128, B_LOC], f32, tag="slab",
                                         name=f"dbg2_{m}")
                        nc.vector.tensor_copy(out=dsl, in_=preBN[m])
                        nc.gpsimd.dma_start(out=dv[m], in_=dsl)
                    return  # skip normalize; only the dbg dump matters
                out_parity = None if li == last_li else li % 2
                bn_normalize(lname, bouts, gam_l, bet_l, out_parity)

            if dbg_mode == 4:
                dv = dbg.rearrange("(m p) b -> m p b", p=128)
                dc = slabp.tile([128, B_LOC], f32, tag="slab", name="dbgc")
                nc.vector.memset(dc, 3.25)
                nc.gpsimd.dma_start(out=dv[0], in_=dc)
                dc2 = slabp.tile([128, B_LOC], f32, tag="slab", name="dbgc2")
                nc.vector.memset(dc2, 7.0)
                nc.vector.tensor_copy(out=dc2[:, 0:128], in_=ones128)
                nc.gpsimd.dma_start(out=dv[1], in_=dc2)
            if dbg_mode == 3:
                dv = dbg.rearrange("(m p) b -> m p b", p=128)
                d0 = slabp.tile([128, B_LOC], f32, tag="slab", name="dbginv")
                nc.vector.memset(d0, 0.0)
                nc.vector.tensor_copy(out=d0[:, 0:1], in_=inv_in)
                nc.vector.tensor_copy(out=d0[:, 1:2], in_=tot_in)
                nc.gpsimd.dma_start(out=dv[0], in_=d0)
                wdbg = wmp.tile([128, KT_IN, 128], bf16, tag="wmi",
                                name="wmidbg", bufs=2)
                nc.sync.dma_start(out=wdbg, in_=tw_in[0])
                d1 = slabp.tile([128, B_LOC], f32, tag="slab", name="dbgtw")
                nc.vector.memset(d1, 0.0)
                nc.vector.tensor_copy(
                    out=d1[:, 0:896],
                    in_=wdbg.rearrange("p t c -> p (t c)"))
                nc.gpsimd.dma_start(out=dv[1], in_=d1)
                d2 = slabp.tile([128, B_LOC], f32, tag="slab", name="dbgx")
                nc.vector.tensor_copy(out=d2, in_=xt[0])
                nc.gpsimd.dma_start(out=dv[2], in_=d2)

            # ================= run all layers ===============================
            run_layer(0, "L0", gam_sb[0], bet_sb[0])
            pump(len(bg))
            ar2()
            if dbg_mode:
                dv = dbg.rearrange("(m p) b -> m p b", p=128)
                for m in range(MT):
                    dsl = slabp.tile([128, B_LOC], f32, tag="slab",
                                     name=f"dbg{m}")
                    nc.vector.tensor_copy(out=dsl, in_=preBN[m])
                    nc.gpsimd.dma_start(out=dv[m], in_=dsl)
            for k in range(n_mid_eff):
                if k + 1 < n_mid_eff:
                    emit_tern_hid(k + 1, invs[k + 1])
                else:
                    bg.append(emit_tern_out)
                run_layer(k + 1, f"H{k}", gam_sb[k + 1], bet_sb[k + 1])
            if n_mid_eff == 0:
                bg.append(emit_tern_out)
            pump(len(bg))

            # ================= output layer + log-softmax ===================
            if dbg_mode == 5:
                fz = small.tile([10, B_LOC], f32, tag="opre", name="fz5")
                nc.vector.memset(fz, 0.0)
                nc.vector.tensor_copy(out=fz[:, 0:1], in_=inv_in[0:10, :])
                nc.vector.tensor_copy(out=fz[:, 1:2], in_=tot_in[0:10, :])
                wdbg = wmp.tile([128, KT_IN, 128], bf16, tag="wmi",
                                name="wmidbg", bufs=2)
                nc.sync.dma_start(out=wdbg, in_=tw_in[0])
                nc.vector.tensor_copy(
                    out=fz[:, 4:900],
                    in_=wdbg[0:10].rearrange("p t c -> p (t c)"))
                nc.gpsimd.dma_start(out=out[:], in_=fz)
            wmo = wmp.tile([128, KT_H, 16], bf16, tag="wmo", name="wmo",
                           bufs=1)
            nc.sync.dma_start(out=wmo,
                              in_=tw_out.rearrange("(t p) c -> p t c", p=128))
            pso = psum.tile([10, B_LOC], f32, tag="mm", name="pso")
            for n in range(2):
                for t in range(KT_H):
                    nc.tensor.matmul(
                        pso[:, n * 512:(n + 1) * 512],
                        wmo[:, t, 0:10],
                        preBN[t][:, n * 512:(n + 1) * 512],
                        start=(t == 0), stop=(t == KT_H - 1))
            S1o = stats.tile([10, 1], f32, tag="s1o")
            S2o = stats.tile([10, 1], f32, tag="s2o")
            opre = small.tile([10, B_LOC], f32, tag="opre")
            nc.vector.tensor_scalar(out=opre, in0=pso, scalar1=1.0,
                                    scalar2=None, op0=OP.mult, op1=OP.add,
                                    accum_out=S1o)
            sjo = sqp.tile([10, B_LOC], bf16, tag="sq", name="sqo")
            nc.scalar.activation(out=sjo, in_=pso, func=AF.Square,
                                 accum_out=S2o)
            bno_in = dram.tile([10, 2], f32)
            bno_out = dram.tile([10, 2], f32, addr_space="Shared")
            s12o = stats.tile([10, 2], f32, tag="s12o")
            nc.vector.tensor_copy(out=s12o[:, 0:1], in_=S1o)
            nc.vector.tensor_copy(out=s12o[:, 1:2], in_=S2o)
            nc.gpsimd.dma_start(out=bno_in, in_=s12o)
            nc.gpsimd.collective_compute(
                "AllReduce", OP.add, replica_groups=RG,
                ins=[bno_in.opt()], outs=[bno_out.opt()])
            sgo = stats.tile([10, 2], f32, tag="sgo")
            nc.gpsimd.dma_start(out=sgo, in_=bno_out)
            meano = stats.tile([10, 1], f32, tag="meano")
            nc.vector.tensor_scalar_mul(meano, sgo[:, 0:1], 1.0 / B)
            ex2o = stats.tile([10, 1], f32, tag="ex2o")
            nc.vector.tensor_scalar_mul(ex2o, sgo[:, 1:2], 1.0 / B)
            msqo = stats.tile([10, 1], f32, tag="msqo")
            nc.vector.tensor_tensor(out=msqo, in0=meano, in1=meano, op=OP.mult)
            varo = stats.tile([10, 1], f32, tag="varo")
            nc.vector.tensor_tensor(out=varo, in0=ex2o, in1=msqo,
                                    op=OP.subtract)
            sdo = stats.tile([10, 1], f32, tag="sdo")
            nc.scalar.activation(out=sdo, in_=varo, func=AF.Sqrt,
                                 bias=epsb[0:10, :])
            rso = stats.tile([10, 1], f32, tag="rso")
            nc.vector.reciprocal(out=rso, in_=sdo)
            sclo = stats.tile([10, 1], f32, tag="sclo")
            nc.vector.tensor_tensor(out=sclo, in0=rso, in1=go_sb, op=OP.mult)
            mso = stats.tile([10, 1], f32, tag="mso")
            nc.vector.tensor_tensor(out=mso, in0=meano, in1=sclo, op=OP.mult)
            biao = stats.tile([10, 1], f32, tag="biao")
            nc.vector.tensor_tensor(out=biao, in0=bo_sb, in1=mso,
                                    op=OP.subtract)
            onorm = small.tile([10, B_LOC], f32, tag="onorm")
            nc.scalar.activation(out=onorm, in_=opre, func=AF.Identity,
                                 bias=biao, scale=sclo)
            esb = small.tile([10, B_LOC], f32, tag="esb")
            nc.scalar.activation(out=esb, in_=onorm, func=AF.Exp)
            csp = psum.tile([1, B_LOC], f32, tag="cs", bufs=1)
            for n in range(2):
                nc.tensor.matmul(csp[:, n * 512:(n + 1) * 512], ones10,
                                 esb[:, n * 512:(n + 1) * 512],
                                 start=True, stop=True)
            lsb = small.tile([1, B_LOC], f32, tag="lsb")
            nc.scalar.activation(out=lsb, in_=csp, func=AF.Ln)
            psb = psum.tile([10, B_LOC], f32, tag="mm", name="psb")
            for n in range(2):
                nc.tensor.matmul(psb[:, n * 512:(n + 1) * 512], onesr[:, 0:10],
                                 lsb[:, n * 512:(n + 1) * 512],
                                 start=True, stop=True)
            fout = small.tile([10, B_LOC], f32, tag="esb", name="fout")
            nc.vector.tensor_tensor(out=fout, in0=onorm, in1=psb,
                                    op=OP.subtract)
            if dbg_mode != 5:
                nc.gpsimd.dma_start(out=out[:], in_=fout)

    nc.compile()
    return nc


def _get_program():
    if "nc" not in _cache:
        _cache["nc"] = _build()
    return _cache["nc"]


def kernel(x, W_in, b_in, W_hid, b_hid, W_out, b_out, gamma, beta,
           gamma_out, beta_out):
    from concourse.bass_utils import run_bass_kernel_spmd

    nc = _get_program()

    x = np.asarray(x, dtype=np.float32).reshape(B, KIN)
    # layout-only host prep (transpose + zero-pad + shard)
    xT_full = np.zeros((KIN_PAD, B), dtype=np.float32)
    xT_full[:KIN] = x.T
    winT_full = np.zeros((KIN_PAD, HID), dtype=np.float32)
    winT_full[:KIN] = np.asarray(W_in, dtype=np.float32).T
    whT_full = np.ascontiguousarray(
        np.asarray(W_hid, dtype=np.float32).transpose(0, 2, 1))
    woT_full = np.zeros((HID, 16), dtype=np.float32)
    woT_full[:, :10] = np.asarray(W_out, dtype=np.float32).T
    gam_np = np.ascontiguousarray(np.asarray(gamma, dtype=np.float32))
    bet_np = np.ascontiguousarray(np.asarray(beta, dtype=np.float32))
    gob_np = np.stack([np.asarray(gamma_out, dtype=np.float32),
                       np.asarray(beta_out, dtype=np.float32)])

    SH_H = HID // N_CORES
    in_maps = []
    for c in range(N_CORES):
        in_maps.append({
            "xT": np.ascontiguousarray(
                xT_full[:, c * B_LOC:(c + 1) * B_LOC]),
            "winT": winT_full,
            "whT": np.ascontiguousarray(
                whT_full[:, c * SH_H:(c + 1) * SH_H, :]),
            "woT": np.ascontiguousarray(
                woT_full[c * SH_H:(c + 1) * SH_H, :]),
            "gam": gam_np,
            "bet": bet_np,
            "gob": gob_np,
        })

    res = run_bass_kernel_spmd(nc, in_maps, core_ids=list(range(N_CORES)))
    if int(os.environ.get("KERNEL_DBG", "0")):
        np.save("/tmp/dbg_h.npy", np.stack(
            [np.asarray(res.results[c]["dbg"]) for c in range(N_CORES)]))
    return np.concatenate(
        [np.ascontiguousarray(res.results[c]["out"].T) for c in range(N_CORES)],
        axis=0)
